# revision 15
# baseline (speedup 1.0000x reference)
"""Trainium2 Bass kernel for nn_MeanEmbedding (fused gather + masked mean).

Strategy:
  out[b] = (1/len_b) * sum_{l < len_b} W[xs[b, l]]
         = (1/len_b) * sum_{v in U} count[v, b] * W[v]

Host builds the set U of unique masked token ids and the (tiny) count
matrix; the device does all heavy HBM work: each unique embedding row is
gathered from HBM exactly once (value-range-sharded across the 8 cores)
and reduced into per-sample sums with PE matmuls (lhsT = counts tile
[128, B], rhs = gathered rows, accumulated in PSUM).  The host sums the
8 per-core partials and divides by the lengths.

Precision/speed: each table row is re-encoded on the host as 3 KiB:
  [ fp16 hi (2048 B) | fp8e4m3 lo' (1024 B) ]
with hi = fp16(W[v]) and lo' = fp8(2^9 * (W[v] - hi)).  The fp16 matmul
uses exact-integer fp16 counts; the fp8 correction matmul uses
counts8 = 2^-9 * count, so the PE product is exactly count * residual
(power-of-two scales are exponent shifts) and both accumulate into the
same fp32 PSUM.  Total representation error ~2^-17 per element —
fp32-grade output — while moving 25% fewer bytes than a plain fp32 row.
"""

import sys

sys.path.insert(0, "/opt/trn_rl_repo")

import ml_dtypes
import numpy as np

FP8 = ml_dtypes.float8_e4m3
LO_SCALE = 512.0  # 2^9

B = 64
L = 2048
V = 50257
D = 1024
N_CORES = 8
P = 128

ROW = 3 * D  # 3 KiB per encoded row, in bytes
VS = -(-V // N_CORES)  # 6283 rows per table shard
V_PAD = VS * N_CORES

_program_cache = {}
LAST_RESULTS = None


def _build_program(R):
    """Build + compile the SPMD Bass program for R gather-tiles per core."""
    import concourse.bass as bass
    import concourse.tile as tile
    from concourse import bacc, mybir

    nc = bacc.Bacc(
        "TRN2",
        target_bir_lowering=False,
        debug=False,
        enable_asserts=False,
        enable_partition_id=False,
        num_devices=N_CORES,
    )
    # encoded table: row v = [fp16 hi | fp8 lo'] as ROW raw bytes
    table = nc.dram_tensor(
        "table", [VS, ROW], mybir.dt.float8e4, kind="ExternalInput"
    ).ap()
    idx = nc.dram_tensor("idx", [P, R], mybir.dt.int32, kind="ExternalInput").ap()
    counts16 = nc.dram_tensor(
        "counts16", [P, R * B], mybir.dt.float16, kind="ExternalInput"
    ).ap()
    counts8 = nc.dram_tensor(
        "counts8", [P, R * B], mybir.dt.float8e4, kind="ExternalInput"
    ).ap()
    out = nc.dram_tensor("out", [B, D], mybir.dt.float32, kind="ExternalOutput").ap()

    with tile.TileContext(nc) as tc:
        with tc.tile_pool(name="meta", bufs=1) as meta, tc.tile_pool(
            name="gath", bufs=8
        ) as gpool, tc.tile_pool(name="acc", bufs=1, space="PSUM") as psum, tc.tile_pool(
            name="outp", bufs=1
        ) as outp:
            idx_sb = meta.tile([P, R], mybir.dt.int32)
            k0 = min(8, R)
            nc.sync.dma_start(idx_sb[:, :k0], idx[:, :k0])
            if k0 < R:
                nc.sync.dma_start(idx_sb[:, k0:], idx[:, k0:])
            c16_sb = meta.tile([P, R * B], mybir.dt.float16)
            c8_sb = meta.tile([P, R * B], mybir.dt.float8e4)
            # split the counts loads so early matmuls only wait on their chunk
            n_chunks = 4
            chunk = -(-R // n_chunks) * B
            for k in range(n_chunks):
                lo_, hi_ = k * chunk, min((k + 1) * chunk, R * B)
                if lo_ < hi_:
                    nc.sync.dma_start(c16_sb[:, lo_:hi_], counts16[:, lo_:hi_])
                    nc.sync.dma_start(c8_sb[:, lo_:hi_], counts8[:, lo_:hi_])

            acc0 = psum.tile([B, 512], mybir.dt.float32)
            acc1 = psum.tile([B, 512], mybir.dt.float32)
            for t in range(R):
                g = gpool.tile([P, ROW], mybir.dt.float8e4, tag="g")
                nc.gpsimd.indirect_dma_start(
                    out=g[:],
                    out_offset=None,
                    in_=table[:],
                    in_offset=bass.IndirectOffsetOnAxis(
                        ap=idx_sb[:, t : t + 1], axis=0
                    ),
                )
                gh = g[:].bitcast(mybir.dt.float16)  # [P, 3*D//2]; hi = [:D]
                l16 = c16_sb[:, t * B : (t + 1) * B]
                l8 = c8_sb[:, t * B : (t + 1) * B]
                first, last = t == 0, t == R - 1
                nc.tensor.matmul(
                    out=acc0[:], lhsT=l16, rhs=gh[:, 0:512],
                    start=first, stop=False,
                )
                nc.tensor.matmul(
                    out=acc0[:], lhsT=l8, rhs=g[:, 2 * D : 2 * D + 512],
                    start=False, stop=last,
                )
                nc.tensor.matmul(
                    out=acc1[:], lhsT=l16, rhs=gh[:, 512:1024],
                    start=first, stop=False,
                )
                nc.tensor.matmul(
                    out=acc1[:], lhsT=l8, rhs=g[:, 2 * D + 512 : 3 * D],
                    start=False, stop=last,
                )
            res = outp.tile([B, D], mybir.dt.float32)
            nc.vector.tensor_copy(res[:, 0:512], acc0[:])
            nc.scalar.copy(res[:, 512:1024], acc1[:])
            nc.sync.dma_start(out[:], res[:])

    nc.compile()
    return nc


def _get_program(R):
    if R not in _program_cache:
        _program_cache[R] = _build_program(R)
    return _program_cache[R]


def _encode_table(W):
    """[V_PAD, ROW] fp8-typed raw bytes: row v = [fp16(W[v]) | fp8(2^9 res)]."""
    enc = np.zeros((V_PAD, ROW), dtype=np.uint8)
    hi = W.astype(np.float16)
    res = W - hi.astype(np.float32)
    lo = (res * LO_SCALE).astype(FP8)
    enc[:V, : 2 * D] = hi.view(np.uint8)
    enc[:V, 2 * D :] = lo.view(np.uint8)
    return enc.view(FP8)


def kernel(xs, xs_len, embed_weight):
    global LAST_RESULTS
    import os
    from concourse import bass_utils

    xs = np.asarray(xs)
    xs_len = np.asarray(xs_len)
    W = np.ascontiguousarray(np.asarray(embed_weight, dtype=np.float32))
    assert xs.shape == (B, L) and W.shape == (V, D)

    # ---- host index preprocessing (O(B*L)) ----
    mask = np.arange(L)[None, :] < xs_len.astype(np.int64)[:, None]
    toks = xs[mask].astype(np.int64)
    samp = np.broadcast_to(np.arange(B)[:, None], (B, L))[mask]
    U, inv = np.unique(toks, return_inverse=True)
    nU = len(U)
    cnt = np.bincount(inv * B + samp, minlength=nU * B).reshape(nU, B)
    # counts ride as fp16 (exact integers up to 2048 = L, the max possible)
    assert cnt.max() <= 2048

    # split unique ids by value range -> core c owns table rows [c*VS, (c+1)*VS)
    shard_of = U // VS
    start = np.searchsorted(shard_of, np.arange(N_CORES), side="left")
    end = np.searchsorted(shard_of, np.arange(N_CORES), side="right")
    n_per_core = end - start
    R = max(1, -(-int(n_per_core.max()) // P))
    Npad = R * P

    Wenc = _encode_table(W)

    in_maps = []
    for c in range(N_CORES):
        lo, hi = int(start[c]), int(end[c])
        n = hi - lo
        idx_c = np.zeros(Npad, np.int32)
        cnt_c = np.zeros((Npad, B), np.float32)
        if n > 0:
            idx_c[:n] = (U[lo:hi] - c * VS).astype(np.int32)
            idx_c[n:] = idx_c[n - 1]
            cnt_c[:n] = cnt[lo:hi]
        idx_pr = np.ascontiguousarray(idx_c.reshape(R, P).T)  # [P, R]
        cnt_prb = np.ascontiguousarray(
            cnt_c.reshape(R, P, B).transpose(1, 0, 2).reshape(P, R * B)
        )  # [P, R*B] fp32
        in_maps.append(
            {
                "table": np.ascontiguousarray(Wenc[c * VS : (c + 1) * VS]),
                "idx": idx_pr,
                "counts16": cnt_prb.astype(np.float16),
                "counts8": (cnt_prb / LO_SCALE).astype(FP8),
            }
        )

    nc = _get_program(R)
    trace = bool(os.environ.get("MEANEMB_TRACE"))
    LAST_RESULTS = bass_utils.run_bass_kernel_spmd(
        nc, in_maps, core_ids=list(range(N_CORES)), trace=trace
    )

    partial = np.stack([LAST_RESULTS.results[c]["out"] for c in range(N_CORES)])
    total = partial.sum(axis=0)
    out = total / xs_len.astype(np.float32)[:, None]
    return out.astype(np.float32)


# revision 16
# speedup vs baseline: 1.1671x; 1.1671x over previous
"""Trainium2 Bass kernel for nn_MeanEmbedding (fused gather + masked mean).

Strategy:
  out[b] = (1/len_b) * sum_{l < len_b} W[xs[b, l]]
         = (1/len_b) * sum_{v in U} count[v, b] * W[v]

Host builds the set U of unique masked token ids and the (tiny) count
matrix; the device does all heavy HBM work: each unique embedding row is
gathered from HBM exactly once (value-range-sharded across the 8 cores)
and reduced into per-sample sums with PE matmuls (lhsT = counts tile
[128, B], rhs = gathered rows, accumulated in PSUM).  The host sums the
8 per-core partials and divides by the lengths.

Precision/speed: the table is re-encoded on the host as an interleaved
hi/lo bf16 pair per row (hi = bf16(W), lo = bf16(W - hi)), so each
gathered row is still 4 KiB and the PE runs 1-cycle/row bf16 matmuls
(hi and lo both accumulate into the same fp32 PSUM).  The hi/lo split
keeps ~2^-17 relative representation error — fp32-grade output.
"""

import sys

sys.path.insert(0, "/opt/trn_rl_repo")

import ml_dtypes
import numpy as np

BF16 = ml_dtypes.bfloat16

B = 64
L = 2048
V = 50257
D = 1024
N_CORES = 8
P = 128

VS = -(-V // N_CORES)  # 6283 rows per table shard
V_PAD = VS * N_CORES

_program_cache = {}
LAST_RESULTS = None


def _build_program(R):
    """Build + compile the SPMD Bass program for R gather-tiles per core."""
    import concourse.bass as bass
    import concourse.tile as tile
    from concourse import bacc, mybir

    nc = bacc.Bacc(
        "TRN2",
        target_bir_lowering=False,
        debug=False,
        enable_asserts=False,
        enable_partition_id=False,
        num_devices=N_CORES,
    )
    # interleaved hi/lo bf16 table: row v = [hi(W[v]), lo(W[v])], 2*D bf16
    table = nc.dram_tensor(
        "table", [VS, 2 * D], mybir.dt.bfloat16, kind="ExternalInput"
    ).ap()
    idx = nc.dram_tensor("idx", [P, R], mybir.dt.int32, kind="ExternalInput").ap()
    counts = nc.dram_tensor(
        "counts", [P, R * B], mybir.dt.bfloat16, kind="ExternalInput"
    ).ap()
    out = nc.dram_tensor("out", [B, D], mybir.dt.float32, kind="ExternalOutput").ap()

    with tile.TileContext(nc) as tc:
        with tc.tile_pool(name="meta", bufs=1) as meta, tc.tile_pool(
            name="gath", bufs=8
        ) as gpool, tc.tile_pool(name="acc", bufs=1, space="PSUM") as psum, tc.tile_pool(
            name="outp", bufs=1
        ) as outp:
            idx_sb = meta.tile([P, R], mybir.dt.int32)
            k0 = min(8, R)
            nc.sync.dma_start(idx_sb[:, :k0], idx[:, :k0])
            if k0 < R:
                nc.sync.dma_start(idx_sb[:, k0:], idx[:, k0:])
            counts_sb = meta.tile([P, R * B], mybir.dt.bfloat16)
            # split the counts load so early matmuls only wait on their chunk
            n_chunks = 4
            chunk = -(-R // n_chunks) * B
            for k in range(n_chunks):
                lo_, hi_ = k * chunk, min((k + 1) * chunk, R * B)
                if lo_ < hi_:
                    nc.sync.dma_start(counts_sb[:, lo_:hi_], counts[:, lo_:hi_])

            acc0 = psum.tile([B, 512], mybir.dt.float32)
            acc1 = psum.tile([B, 512], mybir.dt.float32)
            for t in range(R):
                g = gpool.tile([P, 2 * D], mybir.dt.bfloat16, tag="g")
                nc.gpsimd.indirect_dma_start(
                    out=g[:],
                    out_offset=None,
                    in_=table[:],
                    in_offset=bass.IndirectOffsetOnAxis(
                        ap=idx_sb[:, t : t + 1], axis=0
                    ),
                )
                lhsT = counts_sb[:, t * B : (t + 1) * B]
                first, last = t == 0, t == R - 1
                # cols 0:1024 = hi, 1024:2048 = lo; both accumulate
                nc.tensor.matmul(
                    out=acc0[:], lhsT=lhsT, rhs=g[:, 0:512],
                    start=first, stop=False,
                )
                nc.tensor.matmul(
                    out=acc0[:], lhsT=lhsT, rhs=g[:, 1024:1536],
                    start=False, stop=last,
                )
                nc.tensor.matmul(
                    out=acc1[:], lhsT=lhsT, rhs=g[:, 512:1024],
                    start=first, stop=False,
                )
                nc.tensor.matmul(
                    out=acc1[:], lhsT=lhsT, rhs=g[:, 1536:2048],
                    start=False, stop=last,
                )
            res = outp.tile([B, D], mybir.dt.float32)
            nc.vector.tensor_copy(res[:, 0:512], acc0[:])
            nc.scalar.copy(res[:, 512:1024], acc1[:])
            nc.sync.dma_start(out[:], res[:])

    nc.compile()
    return nc


def _get_program(R):
    if R not in _program_cache:
        _program_cache[R] = _build_program(R)
    return _program_cache[R]


def _hilo_table(W):
    """[V_PAD, 2D] bf16: row v = [bf16(W[v]), bf16(W[v] - fp32(bf16(W[v])))]."""
    Wb = np.zeros((V_PAD, 2 * D), dtype=BF16)
    hi = W.astype(BF16)
    Wb[:V, :D] = hi
    Wb[:V, D:] = (W - hi.astype(np.float32)).astype(BF16)
    return Wb


def kernel(xs, xs_len, embed_weight):
    global LAST_RESULTS
    import os
    from concourse import bass_utils

    xs = np.asarray(xs)
    xs_len = np.asarray(xs_len)
    W = np.ascontiguousarray(np.asarray(embed_weight, dtype=np.float32))
    assert xs.shape == (B, L) and W.shape == (V, D)

    # ---- host index preprocessing (O(B*L)) ----
    mask = np.arange(L)[None, :] < xs_len.astype(np.int64)[:, None]
    toks = xs[mask].astype(np.int64)
    samp = np.broadcast_to(np.arange(B)[:, None], (B, L))[mask]
    U, inv = np.unique(toks, return_inverse=True)
    nU = len(U)
    cnt = np.bincount(inv * B + samp, minlength=nU * B).reshape(nU, B)
    # counts ride as bf16, exact only for integers <= 256; if any count is
    # larger (essentially impossible for random data), split that unique row
    # into several duplicate entries whose counts are each <= 256.
    if cnt.max() > 256:
        reps = -(-int(cnt.max()) // 256)
        U_l, cnt_l = [U], [np.minimum(cnt, 256)]
        rem = cnt - cnt_l[0]
        for _ in range(1, reps):
            rows = np.where(rem.max(axis=1) > 0)[0]
            take = np.minimum(rem[rows], 256)
            U_l.append(U[rows])
            cnt_l.append(take)
            rem[rows] -= take
        U = np.concatenate(U_l)
        cnt = np.concatenate(cnt_l, axis=0)
        order = np.argsort(U, kind="stable")
        U, cnt = U[order], cnt[order]
        nU = len(U)
    assert cnt.max() <= 256

    # split unique ids by value range -> core c owns table rows [c*VS, (c+1)*VS)
    shard_of = U // VS
    start = np.searchsorted(shard_of, np.arange(N_CORES), side="left")
    end = np.searchsorted(shard_of, np.arange(N_CORES), side="right")
    n_per_core = end - start
    R = max(1, -(-int(n_per_core.max()) // P))
    Npad = R * P

    Wb = _hilo_table(W)

    in_maps = []
    for c in range(N_CORES):
        lo, hi = int(start[c]), int(end[c])
        n = hi - lo
        idx_c = np.zeros(Npad, np.int32)
        cnt_c = np.zeros((Npad, B), np.float32)
        if n > 0:
            idx_c[:n] = (U[lo:hi] - c * VS).astype(np.int32)
            idx_c[n:] = idx_c[n - 1]
            cnt_c[:n] = cnt[lo:hi]
        idx_pr = np.ascontiguousarray(idx_c.reshape(R, P).T)  # [P, R]
        cnt_prb = np.ascontiguousarray(
            cnt_c.reshape(R, P, B).transpose(1, 0, 2).reshape(P, R * B)
        ).astype(BF16)  # [P, R*B]
        in_maps.append(
            {
                "table": np.ascontiguousarray(Wb[c * VS : (c + 1) * VS]),
                "idx": idx_pr,
                "counts": cnt_prb,
            }
        )

    nc = _get_program(R)
    trace = bool(os.environ.get("MEANEMB_TRACE"))
    LAST_RESULTS = bass_utils.run_bass_kernel_spmd(
        nc, in_maps, core_ids=list(range(N_CORES)), trace=trace
    )

    partial = np.stack([LAST_RESULTS.results[c]["out"] for c in range(N_CORES)])
    total = partial.sum(axis=0)
    out = total / xs_len.astype(np.float32)[:, None]
    return out.astype(np.float32)
